# revision 12
# baseline (speedup 1.0000x reference)
"""Fused cross-modal attention (concat two QKV streams along sequence, full
softmax attention) on 8 Trainium2 NeuronCores.

Sharding: data-parallel over (batch b, query-half h) -> 8 shards. Each core
computes attention for 2048 queries against the fused 4096-key sequence.

v2: quadrant-paired PE matmuls + gapless Scalar exp stream.

The Scalar (ACT) engine is the hard floor: 2048x4096 exps per core at
1 elem/cycle/lane @1.2GHz = (N+352)/1.2 ns per [128, N] tile -> ~73us at
N=1024. Everything else is organized to hide under it:

  - Scores:   sc[128k, 1024] <- two 64-row quadrant matmuls running
              CONCURRENTLY (key tile 2p at PE rows 0-63, tile 2p+1 at rows
              64-127, outputs in different PSUM banks).
  - PV:       contraction over 128 keys split into two 64-row halves at PE
              row-groups 0/64, accumulating concurrently into acc_lo/acc_hi
              (separate PSUM banks); summed once per query block on DVE.
  - Per round (2 key tiles x 512 queries): PE ~3 matmul slots (~650ns warm,
    ~1300 cold) vs Scalar 1146ns -> Scalar binds even if HAM throttles PE.
  - Emission order pipelines scores(r+1) BEFORE pv(r) so the ACT stream
    never waits on the PE's in-order stall behind exp(r).
  - Input transposes (fp32 PE-transpose pairs to quadrant row bases) are
    emitted just-in-time inside early rounds so the first ACT fires ~4us in.

Layouts (key/query order permuted identically for K and V -> softmax
invariant; query permutation undone by the output DMA access pattern):
  kt2[0:64, s, :]  = d-major key tile 2s,  kt2[64:128, s, :] = tile 2s+1
  qt2[0:64, i, :]  = d-major query tile i, duplicated at partitions 64-127
  v1[p, i, 0:64]   = V row p*32+i, v1[:, :, 64] = 1.0 (softmax denominator
                     lands in acc partition 64 for free)
"""

import numpy as np

import concourse.bass as bass
import concourse.tile as tile
from concourse import mybir
from concourse.bacc import Bacc
from concourse.bass_utils import run_bass_kernel_spmd
from concourse.masks import make_identity

F32 = mybir.dt.float32
F32R = mybir.dt.float32r

B, S, D = 4, 2048, 64
S2 = 2 * S  # fused sequence length 4096
NCORES = 8
QSH = 2048  # queries per core
KT = S2 // 128  # 32 key tiles
QT = QSH // 128  # 16 query tiles
NP = KT // 2  # 16 key-tile pairs
NB = 4  # query blocks of 512
SCALE = 1.0 / float(np.sqrt(D))


def _build():
    nc = Bacc()
    q = nc.declare_dram_parameter("q", [QSH, D], F32, isOutput=False)
    k = nc.declare_dram_parameter("k", [S2, D], F32, isOutput=False)
    v = nc.declare_dram_parameter("v", [S2, D], F32, isOutput=False)
    out = nc.declare_dram_parameter("out", [QSH, D], F32, isOutput=True)

    with tile.TileContext(nc) as tc:
        with (
            tc.tile_pool(name="const", bufs=1) as const_pool,
            tc.tile_pool(name="stage", bufs=1) as stage,
            tc.tile_pool(name="psum", bufs=2, space="PSUM") as psum,
            tc.tile_pool(name="apsum", bufs=1, space="PSUM") as apsum,
            tc.tile_pool(name="exps", bufs=3) as exps,
            tc.tile_pool(name="outp", bufs=3) as outp,
        ):
            ident = const_pool.tile([128, 128], F32)
            make_identity(nc, ident)
            # Touch Exp early so the ~2.7us ACT table load overlaps the
            # input DMAs instead of stalling the first real exp.
            warm = const_pool.tile([128, 1], F32)
            nc.scalar.activation(
                out=warm, in_=ident[:, 0:1],
                func=mybir.ActivationFunctionType.Exp,
            )

            q_ap = q[:].rearrange("(p t) d -> p (t d)", p=128)  # q row p*16+t
            k_ap = k[:].rearrange("(p t) d -> p (t d)", p=128)  # k row p*32+t
            v_ap = v[:].rearrange("(p t) d -> p (t d)", p=128)
            # out[p*16 + g*4 + t, :] <- block g, slot t, partition p
            out_ap = out[:].rearrange("(p g t) d -> g p (t d)", g=NB, t=4)

            # Contiguous staging loads; DMA emission order front-loads what
            # the first rounds need (k chunk 0, all q, v chunk 0).
            NKC = 8  # tiles per chunk
            k_chunks = [None] * 4
            q_chunks = [None] * 2
            v_chunks = [None] * 4

            def load_k(c):
                t = stage.tile([128, NKC, D], F32, tag=f"k_nat{c}")
                nc.sync.dma_start(
                    out=t, in_=k_ap[:, c * NKC * D : (c + 1) * NKC * D]
                )
                k_chunks[c] = t

            def load_q(c):
                t = stage.tile([128, NKC, D], F32, tag=f"q_nat{c}")
                nc.sync.dma_start(
                    out=t, in_=q_ap[:, c * NKC * D : (c + 1) * NKC * D]
                )
                q_chunks[c] = t

            def load_v(c):
                t = stage.tile([128, NKC, D], F32, tag=f"v_nat{c}")
                nc.sync.dma_start(
                    out=t, in_=v_ap[:, c * NKC * D : (c + 1) * NKC * D]
                )
                v_chunks[c] = t

            load_k(0)
            load_q(0)
            load_q(1)
            load_v(0)
            load_k(1)
            load_v(1)
            load_k(2)
            load_v(2)
            load_k(3)
            load_v(3)

            # [V | 1] weight tiles for the PV matmuls.
            v1 = stage.tile([128, KT, D + 1], F32R)
            ones_f32 = stage.tile([128, KT], F32, tag="ones")

            def v1_chunk(c):
                nc.vector.tensor_copy(
                    out=v1[:, c * NKC : (c + 1) * NKC, 0:D], in_=v_chunks[c]
                )

            v1_chunk(0)
            nc.vector.memset(ones_f32, 1.0)
            nc.vector.tensor_copy(out=v1[:, :, D], in_=ones_f32)

            # d-major transposed copies of K (even/odd tiles at partition
            # rows 0-63 / 64-127) and Q (duplicated at both row bases).
            kt2 = stage.tile([128, NP, 128], F32R, tag="kt2")
            qt2 = stage.tile([128, QT, 128], F32R, tag="qt2")

            def kchunk(i):
                return k_chunks[i // NKC][:, i % NKC, :]

            def qchunk(i):
                return q_chunks[i // NKC][:, i % NKC, :]

            # Upper-quadrant placement: transpose-matmuls must output at PSUM
            # partition 0, so transpose to the lo half (plain fp32, as in the
            # baseline), DVE-copy to f32r SBUF (which applies the f32r
            # rounding the PE needs), then hop partitions 0:64 -> 64:128 with
            # a small SBUF->SBUF DMA — the DMA engines are otherwise idle.
            tlo = stage.tile([64, NP, 128], F32R, tag="tlo")

            def tp_pair_k(s):
                pt = psum.tile([128, 128], F32, tag="tp")
                nc.tensor.transpose(pt[0:64, :], kchunk(2 * s), ident)
                nc.vector.tensor_copy(out=kt2[0:64, s, :], in_=pt[0:64, :])
                pt2 = psum.tile([128, 128], F32, tag="tp")
                nc.tensor.transpose(pt2[0:64, :], kchunk(2 * s + 1), ident)
                nc.vector.tensor_copy(out=tlo[:, s, :], in_=pt2[0:64, :])
                nc.sync.dma_start(out=kt2[64:128, s, :], in_=tlo[:, s, :])

            def tp_q(i):
                pt = psum.tile([128, 128], F32, tag="tp")
                nc.tensor.transpose(pt[0:64, :], qchunk(i), ident)
                nc.vector.tensor_copy(out=qt2[0:64, i, :], in_=pt[0:64, :])
                nc.sync.dma_start(
                    out=qt2[64:128, i, :], in_=qt2[0:64, i, :]
                )

            for s in range(4):
                tp_pair_k(s)
            for i in range(4):
                tp_q(i)

            def scores(r):
                j, p = divmod(r, NP)
                sc = psum.tile([128, 1024], F32, tag="sc")
                nc.tensor.matmul(
                    sc[:, 0:512],
                    lhsT=kt2[0:64, p, :],
                    rhs=qt2[0:64, 4 * j : 4 * j + 4, :],
                    start=True,
                    stop=True,
                )
                nc.tensor.matmul(
                    sc[:, 512:1024],
                    lhsT=kt2[64:128, p, :],
                    rhs=qt2[64:128, 4 * j : 4 * j + 4, :],
                    start=True,
                    stop=True,
                )
                return sc

            acc_lo = acc_hi = None
            sc_cur = scores(0)
            for r in range(NB * NP):
                j, p = divmod(r, NP)
                if p == 0:
                    acc_lo = apsum.tile([65, 512], F32, tag="alo")
                    acc_hi = apsum.tile([65, 512], F32, tag="ahi")

                ex = exps.tile([128, 1024], F32R)
                nc.scalar.activation(
                    out=ex,
                    in_=sc_cur,
                    func=mybir.ActivationFunctionType.Exp,
                    scale=SCALE,
                )

                if r + 1 < NB * NP:
                    sc_next = scores(r + 1)
                else:
                    sc_next = None

                # Just-in-time transposes for later rounds (PE slack).
                if r < 12:
                    tp_pair_k(r + 4)
                if j < NB - 1 and 8 <= p < 12:
                    tp_q(4 * (j + 1) + (p - 8))
                if r in (0, 2, 4):
                    v1_chunk(r // 2 + 1)

                # PV: two concurrent 64-row half-contractions per key tile.
                for kk in range(2):
                    i = 2 * p + kk
                    nc.tensor.matmul(
                        acc_lo,
                        lhsT=v1[0:64, i, :],
                        rhs=ex[0:64, kk * 512 : (kk + 1) * 512],
                        start=(p == 0 and kk == 0),
                        stop=(p == NP - 1 and kk == 1),
                        skip_group_check=True,
                    )
                    nc.tensor.matmul(
                        acc_hi,
                        lhsT=v1[64:128, i, :],
                        rhs=ex[64:128, kk * 512 : (kk + 1) * 512],
                        start=(p == 0 and kk == 0),
                        stop=(p == NP - 1 and kk == 1),
                        skip_group_check=True,
                    )

                if p == NP - 1:
                    # Epilogue: combine halves, transpose 128-query chunks,
                    # normalize by the denominator column, store.
                    acc_sb = outp.tile([65, 512], F32, tag="acc_sb")
                    nc.vector.tensor_copy(out=acc_sb, in_=acc_lo)
                    nc.vector.tensor_add(out=acc_sb, in0=acc_sb, in1=acc_hi)
                    ot = outp.tile([128, 4, D], F32, tag="ot")
                    for t in range(4):
                        tr = psum.tile([128, 128], F32, tag="tp")
                        nc.tensor.transpose(
                            tr[:, 0:65],
                            acc_sb[:, t * 128 : (t + 1) * 128],
                            ident[0:65, 0:65],
                        )
                        rc = outp.tile([128, 1], F32, tag="rc")
                        nc.vector.reciprocal(rc, tr[:, 64:65])
                        nc.vector.tensor_scalar_mul(ot[:, t, :], tr[:, 0:D], rc)
                    nc.sync.dma_start(out=out_ap[j], in_=ot)

                sc_cur = sc_next

    nc.finalize()
    return nc


_NC = None


def _get_nc():
    global _NC
    if _NC is None:
        _NC = _build()
    return _NC


def _shard_inputs(Q1, K1, V1, Q2, K2, V2):
    """Core c handles batch c//2, query-half c%2."""
    in_maps = []
    for c in range(NCORES):
        b, h = divmod(c, 2)
        qs = Q1[b] if h == 0 else Q2[b]
        ks = np.concatenate([K1[b], K2[b]], axis=0)
        vs = np.concatenate([V1[b], V2[b]], axis=0)
        in_maps.append(
            {
                "q": np.ascontiguousarray(qs, dtype=np.float32),
                "k": np.ascontiguousarray(ks, dtype=np.float32),
                "v": np.ascontiguousarray(vs, dtype=np.float32),
            }
        )
    return in_maps


def _assemble(results):
    out = np.empty((B, S2, D), dtype=np.float32)
    for c in range(NCORES):
        b, h = divmod(c, 2)
        out[b, h * QSH : (h + 1) * QSH, :] = results[c]["out"]
    return out


def run(inputs, trace=False):
    nc = _get_nc()
    in_maps = _shard_inputs(
        np.asarray(inputs["Q1"]), np.asarray(inputs["K1"]), np.asarray(inputs["V1"]),
        np.asarray(inputs["Q2"]), np.asarray(inputs["K2"]), np.asarray(inputs["V2"]),
    )
    bkr = run_bass_kernel_spmd(nc, in_maps, list(range(NCORES)), trace=trace)
    return _assemble(bkr.results), bkr


def kernel(**inputs) -> np.ndarray:
    out, _ = run(inputs)
    return out
